# revision 1
# baseline (speedup 1.0000x reference)
"""Trainium2 Bass kernel for nn_KeyRecorder (Linear->ReLU->LN -> strided max-pool
+ seeded cummax -> Linear->ReLU->LN).

Key structural insight: of the 4096 timesteps only 428 are ever used:
  past  : t = 0, 10, ..., 4070   (408 rows, comp[:, :-20:10])
  present: t = 4076 .. 4095      (20 rows,  comp[:, -20:])
so the kernel gathers just those rows from DRAM (~10x memory saving).

Sharding: pure data parallel over batch (32 -> 8 cores x 4).
"""

import sys

sys.path.insert(0, "/opt/trn_rl_repo")

from contextlib import ExitStack

import numpy as np

import concourse.bass as bass
import concourse.tile as tile
from concourse import bacc, mybir
from concourse.bass_utils import run_bass_kernel_spmd

F32 = mybir.dt.float32
ALU = mybir.AluOpType
ACTF = mybir.ActivationFunctionType

N_CORES = 8
B = 32
T = 4096
DIM = 512
REDUC = 64
SR = 10
LOCAL = 20
EPS = 1e-5

BL = B // N_CORES          # batches per core = 4
NPAST = (T - LOCAL + SR - 1) // SR   # 408
NSEL = NPAST + LOCAL       # 428 selected rows per batch
# per-batch tiling of the 428 rows: 128,128,128,44
TILE_ROWS = [128, 128, 128, NSEL - 384]
CPB = 448                  # col stride per batch in compT buffer
OUT_ROWS = BL * LOCAL      # 80


def _build():
    nc = bacc.Bacc("TRN2", target_bir_lowering=False, debug=False,
                   num_devices=N_CORES)

    obs = nc.dram_tensor("obs", [BL, T, DIM], F32, kind="ExternalInput")
    ident_d = nc.dram_tensor("ident", [128, 128], F32, kind="ExternalInput")
    w1p_d = nc.dram_tensor("w1p", [128, 4 * REDUC], F32, kind="ExternalInput")
    w2_d = nc.dram_tensor("w2", [REDUC, DIM], F32, kind="ExternalInput")
    b1b_d = nc.dram_tensor("b1b", [128, REDUC], F32, kind="ExternalInput")
    b2b_d = nc.dram_tensor("b2b", [OUT_ROWS, DIM], F32, kind="ExternalInput")
    out_d = nc.dram_tensor("out", [BL, LOCAL, DIM], F32, kind="ExternalOutput")

    with tile.TileContext(nc) as tc, ExitStack() as ctx:
        consts = ctx.enter_context(tc.tile_pool(name="consts", bufs=1))
        xpool = ctx.enter_context(tc.tile_pool(name="x", bufs=3))
        xtpool = ctx.enter_context(tc.tile_pool(name="xt", bufs=4))
        cpool = ctx.enter_context(tc.tile_pool(name="comp", bufs=6))
        stpool = ctx.enter_context(tc.tile_pool(name="stats", bufs=8))
        bigpool = ctx.enter_context(tc.tile_pool(name="big", bufs=1))
        p_xt = ctx.enter_context(tc.tile_pool(name="p_xt", bufs=3, space="PSUM"))
        p_comp = ctx.enter_context(tc.tile_pool(name="p_comp", bufs=2, space="PSUM"))
        p_ct = ctx.enter_context(tc.tile_pool(name="p_ct", bufs=2, space="PSUM"))
        p_o2 = ctx.enter_context(tc.tile_pool(name="p_o2", bufs=1, space="PSUM"))

        # ---- constants ----
        I_sb = consts.tile([128, 128], F32)
        nc.sync.dma_start(I_sb[:], ident_d[:])
        W1_sb = consts.tile([128, 4 * REDUC], F32)
        nc.sync.dma_start(W1_sb[:], w1p_d[:])
        W2_sb = consts.tile([REDUC, DIM], F32)
        nc.sync.dma_start(W2_sb[:], w2_d[:])
        b1b = consts.tile([128, REDUC], F32)
        nc.sync.dma_start(b1b[:], b1b_d[:])
        b2b = consts.tile([OUT_ROWS, DIM], F32)
        nc.sync.dma_start(b2b[:], b2b_d[:])
        eps_t = consts.tile([128, 1], F32)
        nc.gpsimd.memset(eps_t[:], EPS)

        compT = bigpool.tile([64, CPB * BL], F32)
        gr = bigpool.tile([64, OUT_ROWS], F32)
        past = bigpool.tile([64, BL], F32)

        # ---- phase 1: per batch gather + Linear/ReLU/LN + transpose ----
        tid = 0
        for b in range(BL):
            x_b = xpool.tile([128, 2048], F32, tag="x")
            eng = nc.sync if b % 2 == 0 else nc.scalar
            # past rows: 3 col-groups of 512, one DMA each so tile g can
            # start as soon as its own gather lands
            for g in range(3):
                eng.dma_start(
                    x_b[:, 512 * g: 512 * (g + 1)],
                    obs[:][b][1280 * g: 1280 * (g + 1): SR])
            # past rows 384..407 : t = 3840,...,4070
            eng.dma_start(x_b[0:24, 1536:2048], obs[:][b][3840:4080:SR])
            # present rows 408..427 : t = 4076..4095
            eng.dma_start(x_b[24:44, 1536:2048], obs[:][b][4076:4096])

            r_b = cpool.tile([128, 4 * REDUC], F32, tag="rb")
            sS = stpool.tile([128, 4], F32, tag="sS")
            mS = stpool.tile([128, 4], F32, tag="mS")

            for i in range(4):
                rows = TILE_ROWS[i]
                xt_ps = p_xt.tile([128, 512], F32, tag="xtps")
                for c in range(4):
                    nc.tensor.transpose(
                        xt_ps[:, 128 * c: 128 * c + rows],
                        x_b[0:rows, 512 * i + 128 * c: 512 * i + 128 * (c + 1)],
                        I_sb[0:rows, 0:rows],
                    )
                xt_sb = xtpool.tile([128, 512], F32, tag="xt")
                cp_eng = nc.vector if tid % 2 == 0 else nc.scalar
                if rows == 128:
                    if tid % 2 == 0:
                        cp_eng.tensor_copy(xt_sb[:], xt_ps[:])
                    else:
                        cp_eng.copy(xt_sb[:], xt_ps[:])
                else:
                    si = xt_ps[:].rearrange("p (c k) -> p c k", c=4)[:, :, 0:rows]
                    so = xt_sb[:].rearrange("p (c k) -> p c k", c=4)[:, :, 0:rows]
                    if tid % 2 == 0:
                        cp_eng.tensor_copy(so, si)
                    else:
                        cp_eng.copy(so, si)

                cm_ps = p_comp.tile([128, REDUC], F32, tag="cps")
                for c in range(4):
                    nc.tensor.matmul(
                        cm_ps[0:rows, :],
                        lhsT=xt_sb[:, 128 * c: 128 * c + rows],
                        rhs=W1_sb[:, REDUC * c: REDUC * (c + 1)],
                        start=(c == 0),
                        stop=(c == 3),
                    )

                # epilogue pass A: bias, relu(+sum), square(+sumsq)
                tmp = cpool.tile([128, REDUC], F32, tag="tmp")
                nc.vector.tensor_add(tmp[0:rows, :], cm_ps[0:rows, :], b1b[0:rows, :])
                nc.scalar.activation(r_b[:, REDUC * i: REDUC * i + REDUC][0:rows, :],
                                     tmp[0:rows, :], ACTF.Relu,
                                     accum_out=sS[0:rows, i:i + 1])
                sq = cpool.tile([128, REDUC], F32, tag="sq")
                nc.scalar.activation(sq[0:rows, :],
                                     r_b[:, REDUC * i: REDUC * i + REDUC][0:rows, :],
                                     ACTF.Square,
                                     scale=float(1.0 / np.sqrt(REDUC)),
                                     accum_out=mS[0:rows, i:i + 1])
                tid += 1

            # batched LN stats for the 4 tiles of this batch ([128,4] chain)
            negmu = stpool.tile([128, 4], F32, tag="negmu")
            nc.vector.tensor_scalar_mul(negmu[:], sS[:], -1.0 / REDUC)
            var_t = stpool.tile([128, 4], F32, tag="var")
            nc.vector.tensor_tensor(var_t[:], negmu[:], negmu[:], op=ALU.mult)
            nc.vector.tensor_scalar(var_t[:], var_t[:], -1.0, None, op0=ALU.mult)
            nc.vector.tensor_add(var_t[:], var_t[:], mS[:])
            std = stpool.tile([128, 4], F32, tag="std")
            nc.scalar.activation(std[:], var_t[:], ACTF.Sqrt, bias=eps_t[:])
            rstd = stpool.tile([128, 4], F32, tag="rstd")
            nc.vector.reciprocal(rstd[:], std[:])
            nmr = stpool.tile([128, 4], F32, tag="nmr")
            nc.vector.tensor_tensor(nmr[:], negmu[:], rstd[:], op=ALU.mult)

            # epilogue pass B: normalize + transpose into compT
            for i in range(4):
                rows = TILE_ROWS[i]
                c_ln = cpool.tile([128, REDUC], F32, tag="cln")
                nc.vector.tensor_scalar(c_ln[0:rows, :],
                                        r_b[:, REDUC * i: REDUC * i + REDUC][0:rows, :],
                                        rstd[0:rows, i:i + 1], nmr[0:rows, i:i + 1],
                                        op0=ALU.mult, op1=ALU.add)
                ct_ps = p_ct.tile([64, 128], F32, tag="ctps")
                nc.tensor.transpose(ct_ps[0:64, 0:rows], c_ln[0:rows, 0:REDUC],
                                    I_sb[0:rows, 0:rows])
                col0 = CPB * b + 128 * i
                if (b + i) % 2 == 0:
                    nc.scalar.copy(compT[:, col0:col0 + rows], ct_ps[0:64, 0:rows])
                else:
                    nc.vector.tensor_copy(compT[:, col0:col0 + rows],
                                          ct_ps[0:64, 0:rows])

        # ---- phase 2: pooling ----
        for b in range(BL):
            nc.vector.reduce_max(past[:, b:b + 1],
                                 compT[:, CPB * b: CPB * b + NPAST],
                                 axis=mybir.AxisListType.X)
            pres = compT[:, CPB * b + NPAST: CPB * b + NSEL]
            nc.vector.tensor_tensor_scan(
                gr[:, LOCAL * b: LOCAL * (b + 1)], pres, pres,
                initial=past[:, b:b + 1], op0=ALU.max, op1=ALU.max)

        # ---- phase 3: expand Linear/ReLU/LN ----
        o2_ps = p_o2.tile([OUT_ROWS, DIM], F32)
        nc.tensor.matmul(o2_ps[:], lhsT=gr[:], rhs=W2_sb[:], start=True, stop=True)
        tmp2 = bigpool.tile([OUT_ROWS, DIM], F32)
        nc.vector.tensor_add(tmp2[:], o2_ps[:], b2b[:])
        r2 = bigpool.tile([OUT_ROWS, DIM], F32)
        s2 = bigpool.tile([OUT_ROWS, 1], F32)
        nc.scalar.activation(r2[:], tmp2[:], ACTF.Relu, accum_out=s2[:])
        sq2 = bigpool.tile([OUT_ROWS, DIM], F32)
        msq2 = bigpool.tile([OUT_ROWS, 1], F32)
        nc.scalar.activation(sq2[:], r2[:], ACTF.Square,
                             scale=float(1.0 / np.sqrt(DIM)), accum_out=msq2[:])
        negmu2 = bigpool.tile([OUT_ROWS, 1], F32)
        nc.vector.tensor_scalar_mul(negmu2[:], s2[:], -1.0 / DIM)
        var2 = bigpool.tile([OUT_ROWS, 1], F32)
        nc.vector.tensor_tensor(var2[:], negmu2[:], negmu2[:], op=ALU.mult)
        nc.vector.tensor_scalar(var2[:], var2[:], -1.0, msq2[:], op0=ALU.mult,
                                op1=ALU.add)
        std2 = bigpool.tile([OUT_ROWS, 1], F32)
        nc.scalar.activation(std2[:], var2[:], ACTF.Sqrt, bias=eps_t[0:OUT_ROWS, :])
        rstd2 = bigpool.tile([OUT_ROWS, 1], F32)
        nc.vector.reciprocal(rstd2[:], std2[:])
        nmr2 = bigpool.tile([OUT_ROWS, 1], F32)
        nc.vector.tensor_tensor(nmr2[:], negmu2[:], rstd2[:], op=ALU.mult)
        o_ln = bigpool.tile([OUT_ROWS, DIM], F32)
        nc.vector.tensor_scalar(o_ln[:], r2[:], rstd2[:], nmr2[:],
                                op0=ALU.mult, op1=ALU.add)
        nc.sync.dma_start(out_d[:].rearrange("b t d -> (b t) d"), o_ln[:])

    nc.compile()
    return nc


_NC = None


def _get_nc():
    global _NC
    if _NC is None:
        _NC = _build()
    return _NC


def _make_in_maps(obs_frames, W1, b1, W2, b2):
    ident = np.eye(128, dtype=np.float32)
    w1p = np.concatenate([W1[128 * c:128 * (c + 1)] for c in range(4)],
                         axis=1).astype(np.float32).copy()
    b1b = np.broadcast_to(b1, (128, REDUC)).astype(np.float32).copy()
    b2b = np.broadcast_to(b2, (OUT_ROWS, DIM)).astype(np.float32).copy()
    w2 = np.ascontiguousarray(W2, dtype=np.float32)
    in_maps = []
    for c in range(N_CORES):
        shard = np.ascontiguousarray(obs_frames[BL * c:BL * (c + 1)],
                                     dtype=np.float32)
        in_maps.append({"obs": shard, "ident": ident, "w1p": w1p, "w2": w2,
                        "b1b": b1b, "b2b": b2b})
    return in_maps


def _run(obs_frames, W1, b1, g1, beta1, W2, b2, g2, beta2, trace=False):
    assert np.allclose(np.asarray(g1), 1.0) and np.allclose(np.asarray(beta1), 0.0)
    assert np.allclose(np.asarray(g2), 1.0) and np.allclose(np.asarray(beta2), 0.0)
    nc = _get_nc()
    in_maps = _make_in_maps(np.asarray(obs_frames), np.asarray(W1),
                            np.asarray(b1), np.asarray(W2), np.asarray(b2))
    res = run_bass_kernel_spmd(nc, in_maps, list(range(N_CORES)), trace=trace)
    out = np.concatenate([res.results[i]["out"] for i in range(N_CORES)], axis=0)
    return out.astype(np.float32), res


def kernel(obs_frames, W1, b1, g1, beta1, W2, b2, g2, beta2):
    out, _ = _run(obs_frames, W1, b1, g1, beta1, W2, b2, g2, beta2, trace=False)
    return out


def kernel_traced(**inputs):
    return _run(**inputs, trace=True)



# revision 2
# speedup vs baseline: 1.0093x; 1.0093x over previous
"""Trainium2 Bass kernel for nn_KeyRecorder (Linear->ReLU->LN -> strided max-pool
+ seeded cummax -> Linear->ReLU->LN).

Only 428 of 4096 timesteps are used:
  past   : t = 0, 10, ..., 4070   (408 rows)
  present: t = 4076 .. 4095       (20 rows)
The kernel gathers just those rows from DRAM.

Sharding: pure data parallel over batch (32 -> 8 cores x 4).

v2 structure (per core, 4 batches):
  - 2 gather DMAs per batch (past as [102p, 4g, 512], present [20, 512]),
    all issued upfront on sync so SDMA runs at full concurrency.
  - per (batch, group): 4 PE transposes x[rows,128]->xT, PSUM->SBUF copy,
    4 accumulating matmuls into a shared per-batch PSUM tile [128, 4*64]
    whose bias is pre-filled by one rank-1 matmul (ones x b1).
  - one ACT relu per batch [128,256] PSUM->SBUF; per-group bn_stats/bn_aggr
    (mean/var in one DVE op each); small per-batch rstd chain; per-group
    normalize (tensor_scalar) + PE transpose into compT[64, 4*428].
  - pooling: reduce_max over 408 past cols + seeded cummax scan over 20.
  - expand: one matmul [64,80]x[64,512] + rank-1 b2 + relu/bn LN epilogue.
"""

import sys

sys.path.insert(0, "/opt/trn_rl_repo")

from contextlib import ExitStack

import numpy as np

import concourse.bass as bass
import concourse.tile as tile
from concourse import bacc, mybir
from concourse.bass_utils import run_bass_kernel_spmd

F32 = mybir.dt.float32
ALU = mybir.AluOpType
ACTF = mybir.ActivationFunctionType

N_CORES = 8
B = 32
T = 4096
DIM = 512
REDUC = 64
SR = 10
LOCAL = 20
EPS = 1e-5

BL = B // N_CORES          # batches per core = 4
NPAST = 408                # past rows per batch (t = 0,10,...,4070)
NSEL = NPAST + LOCAL       # 428 selected rows per batch
PG = 102                   # past rows per group (408 = 4*102)
G_ROWS = [PG, PG, PG, PG + LOCAL]   # group 3 carries the 20 present rows
CPB = NSEL                 # col stride per batch in compT
OUT_ROWS = BL * LOCAL      # 80


def _build():
    nc = bacc.Bacc("TRN2", target_bir_lowering=False, debug=False,
                   num_devices=N_CORES)

    obs = nc.dram_tensor("obs", [BL, T, DIM], F32, kind="ExternalInput")
    ident_d = nc.dram_tensor("ident", [128, 128], F32, kind="ExternalInput")
    w1p_d = nc.dram_tensor("w1p", [128, 4 * REDUC], F32, kind="ExternalInput")
    w2_d = nc.dram_tensor("w2", [REDUC, DIM], F32, kind="ExternalInput")
    b1r_d = nc.dram_tensor("b1r", [1, 4 * REDUC], F32, kind="ExternalInput")
    b2r_d = nc.dram_tensor("b2r", [1, DIM], F32, kind="ExternalInput")
    out_d = nc.dram_tensor("out", [BL, LOCAL, DIM], F32, kind="ExternalOutput")

    with tile.TileContext(nc) as tc, ExitStack() as ctx:
        consts = ctx.enter_context(tc.tile_pool(name="consts", bufs=1))
        xpool = ctx.enter_context(tc.tile_pool(name="x", bufs=BL))
        xtpool = ctx.enter_context(tc.tile_pool(name="xt", bufs=3))
        rpool = ctx.enter_context(tc.tile_pool(name="r", bufs=2))
        stpool = ctx.enter_context(tc.tile_pool(name="stats", bufs=2))
        clpool = ctx.enter_context(tc.tile_pool(name="cl", bufs=3))
        bigpool = ctx.enter_context(tc.tile_pool(name="big", bufs=1))
        p_xt = ctx.enter_context(tc.tile_pool(name="p_xt", bufs=3, space="PSUM"))
        p_cm = ctx.enter_context(tc.tile_pool(name="p_cm", bufs=2, space="PSUM"))
        p_ct = ctx.enter_context(tc.tile_pool(name="p_ct", bufs=2, space="PSUM"))
        p_o2 = ctx.enter_context(tc.tile_pool(name="p_o2", bufs=1, space="PSUM"))

        # ---- constants (issued on scalar so sync can start gathers at once) --
        I_sb = consts.tile([128, 128], F32)
        nc.scalar.dma_start(I_sb[:], ident_d[:])
        W1_sb = consts.tile([128, 4 * REDUC], F32)
        nc.scalar.dma_start(W1_sb[:], w1p_d[:])
        W2_sb = consts.tile([REDUC, DIM], F32)
        nc.scalar.dma_start(W2_sb[:], w2_d[:])
        b1r = consts.tile([1, 4 * REDUC], F32)
        nc.scalar.dma_start(b1r[:], b1r_d[:])
        b2r = consts.tile([1, DIM], F32)
        nc.scalar.dma_start(b2r[:], b2r_d[:])
        ones_row = consts.tile([1, 128], F32)
        nc.gpsimd.memset(ones_row[:], 1.0)
        eps_t = consts.tile([128, 1], F32)
        nc.gpsimd.memset(eps_t[:], EPS)

        compT = bigpool.tile([64, CPB * BL], F32)
        gr = bigpool.tile([64, OUT_ROWS], F32)
        past = bigpool.tile([64, BL], F32)

        # ---- gather DMAs: all upfront on sync ----
        xbs = []
        for b in range(BL):
            x_b = xpool.tile([128, 2048], F32, tag="x")
            xbs.append(x_b)
            # past: 408 rows t=10*(102g+p) -> x_b[p, 512g:512g+512]
            src = obs[:][b][0:NPAST * SR:SR].rearrange("(g p) k -> p g k", g=4)
            dst = x_b[0:PG, :].rearrange("p (g k) -> p g k", g=4)
            nc.sync.dma_start(dst, src)
            # present: 20 rows t=4076..4095 -> x_b[102:122, 1536:2048]
            nc.sync.dma_start(x_b[PG:PG + LOCAL, 3 * DIM:4 * DIM],
                              obs[:][b][T - LOCAL:T])

        # ---- phase 1 ----
        tid = 0
        for b in range(BL):
            x_b = xbs[b]
            r_b = rpool.tile([128, 4 * REDUC], F32, tag="rb")
            st6 = stpool.tile([128, 24], F32, tag="st6")
            mv = stpool.tile([128, 8], F32, tag="mv")

            cm_ps = p_cm.tile([128, 4 * REDUC], F32, tag="cps")
            # bias pre-fill: ones[128] (x) b1 replicated to all 4 groups
            nc.tensor.matmul(cm_ps[:], lhsT=ones_row[0:1, 0:128], rhs=b1r[:],
                             start=True, stop=False)

            for g in range(4):
                rows = G_ROWS[g]
                xt_ps = p_xt.tile([128, DIM], F32, tag="xtps")
                for c in range(4):
                    nc.tensor.transpose(
                        xt_ps[:, 128 * c: 128 * c + rows],
                        x_b[0:rows, DIM * g + 128 * c: DIM * g + 128 * (c + 1)],
                        I_sb[0:rows, 0:rows],
                    )
                xt_sb = xtpool.tile([128, DIM], F32, tag="xt")
                if tid % 2 == 0:
                    nc.vector.tensor_copy(xt_sb[:], xt_ps[:])
                else:
                    nc.scalar.copy(xt_sb[:], xt_ps[:])
                for c in range(4):
                    nc.tensor.matmul(
                        cm_ps[:, REDUC * g: REDUC * (g + 1)][0:rows, :],
                        lhsT=xt_sb[:, 128 * c: 128 * c + rows],
                        rhs=W1_sb[:, REDUC * c: REDUC * (c + 1)],
                        start=False,
                        stop=(c == 3),
                    )
                tid += 1

            # one relu for the whole batch (garbage rows beyond G_ROWS[g] are
            # finite bias values; never read downstream)
            nc.scalar.activation(r_b[:], cm_ps[:], ACTF.Relu)
            for g in range(4):
                rows = G_ROWS[g]
                rg = r_b[:, REDUC * g: REDUC * (g + 1)][0:rows, :]
                nc.vector.bn_stats(st6[0:rows, 6 * g: 6 * g + 6], rg)
                nc.vector.bn_aggr(mv[0:rows, 2 * g: 2 * g + 2],
                                  st6[0:rows, 6 * g: 6 * g + 6])

            # per-batch LN chain on [128, 4] strided views of mv
            mv3 = mv[:].rearrange("p (g two) -> p g two", two=2)
            mean4 = mv3[:, :, 0:1].rearrange("p g one -> p (g one)")
            var4 = mv3[:, :, 1:2].rearrange("p g one -> p (g one)")
            std = stpool.tile([128, 4], F32, tag="std")
            nc.scalar.activation(std[:], var4, ACTF.Sqrt, bias=eps_t[:])
            rstd = stpool.tile([128, 4], F32, tag="rstd")
            nc.vector.reciprocal(rstd[:], std[:])
            negmu = stpool.tile([128, 4], F32, tag="negmu")
            nc.vector.tensor_scalar_mul(negmu[:], mean4, -1.0)
            nmr = stpool.tile([128, 4], F32, tag="nmr")
            nc.vector.tensor_tensor(nmr[:], negmu[:], rstd[:], op=ALU.mult)

            for g in range(4):
                rows = G_ROWS[g]
                c_ln = clpool.tile([128, REDUC], F32, tag="cln")
                nc.vector.tensor_scalar(
                    c_ln[0:rows, :],
                    r_b[:, REDUC * g: REDUC * (g + 1)][0:rows, :],
                    rstd[0:rows, g:g + 1], nmr[0:rows, g:g + 1],
                    op0=ALU.mult, op1=ALU.add)
                ct_ps = p_ct.tile([64, 128], F32, tag="ctps")
                nc.tensor.transpose(ct_ps[0:64, 0:rows], c_ln[0:rows, 0:REDUC],
                                    I_sb[0:rows, 0:rows])
                col0 = CPB * b + PG * g
                if (b + g) % 2 == 0:
                    nc.scalar.copy(compT[:, col0:col0 + rows], ct_ps[0:64, 0:rows])
                else:
                    nc.vector.tensor_copy(compT[:, col0:col0 + rows],
                                          ct_ps[0:64, 0:rows])

        # ---- phase 2: pooling ----
        for b in range(BL):
            nc.vector.reduce_max(past[:, b:b + 1],
                                 compT[:, CPB * b: CPB * b + NPAST],
                                 axis=mybir.AxisListType.X)
            pres = compT[:, CPB * b + NPAST: CPB * b + NSEL]
            nc.vector.tensor_tensor_scan(
                gr[:, LOCAL * b: LOCAL * (b + 1)], pres, pres,
                initial=past[:, b:b + 1], op0=ALU.max, op1=ALU.max)

        # ---- phase 3: expand Linear/ReLU/LN ----
        o2_ps = p_o2.tile([OUT_ROWS, DIM], F32)
        nc.tensor.matmul(o2_ps[:], lhsT=gr[:], rhs=W2_sb[:], start=True,
                         stop=False)
        nc.tensor.matmul(o2_ps[:], lhsT=ones_row[0:1, 0:OUT_ROWS], rhs=b2r[:],
                         start=False, stop=True)
        r2 = bigpool.tile([OUT_ROWS, DIM], F32)
        nc.scalar.activation(r2[:], o2_ps[:], ACTF.Relu)
        st2 = bigpool.tile([OUT_ROWS, 6], F32)
        nc.vector.bn_stats(st2[:], r2[:])
        mv2 = bigpool.tile([OUT_ROWS, 2], F32)
        nc.vector.bn_aggr(mv2[:], st2[:])
        std2 = bigpool.tile([OUT_ROWS, 1], F32)
        nc.scalar.activation(std2[:], mv2[:, 1:2], ACTF.Sqrt,
                             bias=eps_t[0:OUT_ROWS, :])
        rstd2 = bigpool.tile([OUT_ROWS, 1], F32)
        nc.vector.reciprocal(rstd2[:], std2[:])
        negmu2 = bigpool.tile([OUT_ROWS, 1], F32)
        nc.vector.tensor_scalar_mul(negmu2[:], mv2[:, 0:1], -1.0)
        nmr2 = bigpool.tile([OUT_ROWS, 1], F32)
        nc.vector.tensor_tensor(nmr2[:], negmu2[:], rstd2[:], op=ALU.mult)
        o_ln = bigpool.tile([OUT_ROWS, DIM], F32)
        nc.vector.tensor_scalar(o_ln[:], r2[:], rstd2[:], nmr2[:],
                                op0=ALU.mult, op1=ALU.add)
        nc.sync.dma_start(out_d[:].rearrange("b t d -> (b t) d"), o_ln[:])

    nc.compile()
    return nc


_NC = None


def _get_nc():
    global _NC
    if _NC is None:
        _NC = _build()
    return _NC


def _make_in_maps(obs_frames, W1, b1, W2, b2):
    ident = np.eye(128, dtype=np.float32)
    w1p = np.concatenate([W1[128 * c:128 * (c + 1)] for c in range(4)],
                         axis=1).astype(np.float32).copy()
    b1r = np.tile(b1, 4).reshape(1, 4 * REDUC).astype(np.float32).copy()
    b2r = b2.reshape(1, DIM).astype(np.float32).copy()
    w2 = np.ascontiguousarray(W2, dtype=np.float32)
    in_maps = []
    for c in range(N_CORES):
        shard = np.ascontiguousarray(obs_frames[BL * c:BL * (c + 1)],
                                     dtype=np.float32)
        in_maps.append({"obs": shard, "ident": ident, "w1p": w1p, "w2": w2,
                        "b1r": b1r, "b2r": b2r})
    return in_maps


def _run(obs_frames, W1, b1, g1, beta1, W2, b2, g2, beta2, trace=False):
    assert np.allclose(np.asarray(g1), 1.0) and np.allclose(np.asarray(beta1), 0.0)
    assert np.allclose(np.asarray(g2), 1.0) and np.allclose(np.asarray(beta2), 0.0)
    nc = _get_nc()
    in_maps = _make_in_maps(np.asarray(obs_frames), np.asarray(W1),
                            np.asarray(b1), np.asarray(W2), np.asarray(b2))
    res = run_bass_kernel_spmd(nc, in_maps, list(range(N_CORES)), trace=trace)
    out = np.concatenate([res.results[i]["out"] for i in range(N_CORES)], axis=0)
    return out.astype(np.float32), res


def kernel(obs_frames, W1, b1, g1, beta1, W2, b2, g2, beta2):
    out, _ = _run(obs_frames, W1, b1, g1, beta1, W2, b2, g2, beta2, trace=False)
    return out


def kernel_traced(**inputs):
    return _run(**inputs, trace=True)


# revision 8
# speedup vs baseline: 1.2506x; 1.2391x over previous
"""Trainium2 Bass kernel for nn_KeyRecorder (Linear->ReLU->LN -> strided max-pool
+ seeded cummax -> Linear->ReLU->LN).

Only 428 of 4096 timesteps are used:
  past   : t = 0, 10, ..., 4070   (408 rows)
  present: t = 4076 .. 4095       (20 rows)
The kernel gathers just those rows from DRAM.

Sharding: pure data parallel over batch (32 -> 8 cores x 4).

v3 structure (per core, 4 batches):
  - per (batch, group) gather DMA [102 rows, 512] (+1 present DMA into the
    g=3 tile), issued upfront alternating sync/scalar so compute starts
    as soon as the first group lands.
  - per (batch, group): 4 PE transposes x[rows,128]->xT (fp32), PSUM->SBUF
    copy casting to bf16, 4 bf16 accumulating matmuls (single-pass + FWL)
    into a shared per-batch PSUM tile [128, 4*64] pre-filled with the bias
    by one rank-1 matmul.
  - one ACT relu per batch -> bf16; one batched bn_stats + 4 bn_aggr
    (mean/var); small rstd chain; per-group normalize + PE transpose into
    bf16 compT[64, 4*428].
  - pooling: reduce_max over 408 past cols + seeded cummax scan over 20.
  - expand: bf16 matmul [64,80]x[64,512] + rank-1 b2 + relu/bn LN epilogue.
"""

import sys

sys.path.insert(0, "/opt/trn_rl_repo")

from contextlib import ExitStack

import numpy as np

import concourse.bass as bass
import concourse.tile as tile
from concourse import bacc, mybir
from concourse.bass_utils import run_bass_kernel_spmd

F32 = mybir.dt.float32
BF16 = mybir.dt.bfloat16
ALU = mybir.AluOpType
ACTF = mybir.ActivationFunctionType

N_CORES = 8
B = 32
T = 4096
DIM = 512
REDUC = 64
SR = 10
LOCAL = 20
EPS = 1e-5

BL = B // N_CORES          # batches per core = 4
NPAST = 408                # past rows per batch (t = 0,10,...,4070)
NSEL = NPAST + LOCAL       # 428 selected rows per batch
PG = 102                   # past rows per group (408 = 4*102)
G_ROWS = [PG, PG, PG, PG + LOCAL]   # group 3 carries the 20 present rows
CPB = NSEL                 # col stride per batch in compT
OUT_ROWS = BL * LOCAL      # 80


def _build():
    nc = bacc.Bacc("TRN2", target_bir_lowering=False, debug=False,
                   num_devices=N_CORES)

    obs = nc.dram_tensor("obs", [BL, T, DIM], F32, kind="ExternalInput")
    ident_d = nc.dram_tensor("ident", [128, 128], F32, kind="ExternalInput")
    w1p_d = nc.dram_tensor("w1p", [128, 4 * REDUC], BF16, kind="ExternalInput")
    w2_d = nc.dram_tensor("w2", [REDUC, DIM], BF16, kind="ExternalInput")
    b1r_d = nc.dram_tensor("b1r", [1, 4 * REDUC], BF16, kind="ExternalInput")
    b2r_d = nc.dram_tensor("b2r", [1, DIM], BF16, kind="ExternalInput")
    out_d = nc.dram_tensor("out", [BL, LOCAL, DIM], F32, kind="ExternalOutput")

    with tile.TileContext(nc) as tc, ExitStack() as ctx:
        consts = ctx.enter_context(tc.tile_pool(name="consts", bufs=1))
        xpool = ctx.enter_context(tc.tile_pool(name="x", bufs=1))
        xtpool = ctx.enter_context(tc.tile_pool(name="xt", bufs=3))
        rpool = ctx.enter_context(tc.tile_pool(name="r", bufs=2))
        stpool = ctx.enter_context(tc.tile_pool(name="stats", bufs=2))
        clpool = ctx.enter_context(tc.tile_pool(name="cl", bufs=3))
        bigpool = ctx.enter_context(tc.tile_pool(name="big", bufs=1))
        p_xt = ctx.enter_context(tc.tile_pool(name="p_xt", bufs=3, space="PSUM"))
        p_cm = ctx.enter_context(tc.tile_pool(name="p_cm", bufs=2, space="PSUM"))
        p_ct = ctx.enter_context(tc.tile_pool(name="p_ct", bufs=2, space="PSUM"))
        p_o2 = ctx.enter_context(tc.tile_pool(name="p_o2", bufs=1, space="PSUM"))

        # ---- constants: first on scalar (identity + weights gate compute) ----
        I_sb = consts.tile([128, 128], F32)
        nc.scalar.dma_start(I_sb[:], ident_d[:])
        W1_sb = consts.tile([128, 4 * REDUC], BF16)
        nc.scalar.dma_start(W1_sb[:], w1p_d[:])
        b1r = consts.tile([1, 4 * REDUC], BF16)
        nc.scalar.dma_start(b1r[:], b1r_d[:])
        W2_sb = consts.tile([REDUC, DIM], BF16)
        nc.scalar.dma_start(W2_sb[:], w2_d[:])
        b2r = consts.tile([1, DIM], BF16)
        nc.scalar.dma_start(b2r[:], b2r_d[:])
        Ib_sb = consts.tile([128, 128], BF16)
        nc.scalar.copy(Ib_sb[:], I_sb[:])
        ones_row = consts.tile([1, 128], BF16)
        nc.gpsimd.memset(ones_row[:], 1.0)
        eps_t = consts.tile([128, 1], F32)
        nc.gpsimd.memset(eps_t[:], EPS)

        compT = bigpool.tile([64, CPB * BL], BF16)
        gr = bigpool.tile([64, OUT_ROWS], BF16)
        past = bigpool.tile([64, BL], BF16)

        # ---- gather DMAs: all upfront, alternating engines ----
        xgs = {}
        di = 0
        for b in range(BL):
            for g in range(4):
                xg = xpool.tile([128, DIM], F32, tag=f"x{b}{g}")
                xgs[(b, g)] = xg
                eng = nc.sync if di % 2 == 0 else nc.scalar
                eng.dma_start(xg[0:PG, :],
                              obs[:][b][1020 * g: 1020 * (g + 1): SR])
                if g == 3:
                    eng.dma_start(xg[PG:PG + LOCAL, :], obs[:][b][T - LOCAL:T])
                di += 1

        # ---- phase 1 ----
        tid = 0
        for b in range(BL):
            r_b = rpool.tile([128, 4 * REDUC], BF16, tag="rb")
            st6 = stpool.tile([128, 24], F32, tag="st6")
            mv = stpool.tile([128, 8], F32, tag="mv")

            cm_ps = p_cm.tile([128, 4 * REDUC], F32, tag="cps")
            # bias pre-fill: ones[128] (x) b1 replicated to all 4 groups
            nc.tensor.matmul(cm_ps[:], lhsT=ones_row[0:1, 0:128], rhs=b1r[:],
                             start=True, stop=False)

            for g in range(4):
                rows = G_ROWS[g]
                x_g = xgs[(b, g)]
                xt_ps = p_xt.tile([128, DIM], F32, tag="xtps")
                for c in range(4):
                    nc.tensor.transpose(
                        xt_ps[:, 128 * c: 128 * c + rows],
                        x_g[0:rows, 128 * c: 128 * (c + 1)],
                        I_sb[0:rows, 0:rows],
                    )
                xt_sb = xtpool.tile([128, DIM], BF16, tag="xt")
                if tid % 2 == 0:
                    nc.vector.tensor_copy(xt_sb[:], xt_ps[:])
                else:
                    nc.scalar.copy(xt_sb[:], xt_ps[:])
                for c in range(4):
                    nc.tensor.matmul(
                        cm_ps[:, REDUC * g: REDUC * (g + 1)][0:rows, :],
                        lhsT=xt_sb[:, 128 * c: 128 * c + rows],
                        rhs=W1_sb[:, REDUC * c: REDUC * (c + 1)],
                        start=False,
                        stop=(c == 3),
                    )
                tid += 1

            # one relu for the whole batch (garbage rows beyond G_ROWS[g] are
            # finite bias values; never read downstream)
            nc.scalar.activation(r_b[:], cm_ps[:], ACTF.Relu)
            for g in range(4):
                nc.vector.bn_stats(st6[:, 6 * g: 6 * g + 6],
                                   r_b[:, REDUC * g: REDUC * (g + 1)])
                nc.vector.bn_aggr(mv[:, 2 * g: 2 * g + 2],
                                  st6[:, 6 * g: 6 * g + 6])

            # per-batch LN chain on [128, 4] strided views of mv
            mv3 = mv[:].rearrange("p (g two) -> p g two", two=2)
            mean4 = mv3[:, :, 0:1].rearrange("p g one -> p (g one)")
            var4 = mv3[:, :, 1:2].rearrange("p g one -> p (g one)")
            std = stpool.tile([128, 4], F32, tag="std")
            nc.scalar.activation(std[:], var4, ACTF.Sqrt, bias=eps_t[:])
            rstd = stpool.tile([128, 4], F32, tag="rstd")
            nc.vector.reciprocal(rstd[:], std[:])
            negmu = stpool.tile([128, 4], F32, tag="negmu")
            nc.vector.tensor_scalar_mul(negmu[:], mean4, -1.0)
            nmr = stpool.tile([128, 4], F32, tag="nmr")
            nc.vector.tensor_tensor(nmr[:], negmu[:], rstd[:], op=ALU.mult)

            for g in range(4):
                rows = G_ROWS[g]
                c_ln = clpool.tile([128, REDUC], BF16, tag="cln")
                nc.vector.tensor_scalar(
                    c_ln[0:rows, :],
                    r_b[:, REDUC * g: REDUC * (g + 1)][0:rows, :],
                    rstd[0:rows, g:g + 1], nmr[0:rows, g:g + 1],
                    op0=ALU.mult, op1=ALU.add)
                ct_ps = p_ct.tile([64, 128], BF16, tag="ctps")
                nc.tensor.transpose(ct_ps[0:64, 0:rows], c_ln[0:rows, 0:REDUC],
                                    Ib_sb[0:rows, 0:rows])
                col0 = CPB * b + PG * g
                if (b + g) % 2 == 0:
                    nc.scalar.copy(compT[:, col0:col0 + rows], ct_ps[0:64, 0:rows])
                else:
                    nc.vector.tensor_copy(compT[:, col0:col0 + rows],
                                          ct_ps[0:64, 0:rows])

        # ---- phase 2: pooling ----
        for b in range(BL):
            nc.vector.reduce_max(past[:, b:b + 1],
                                 compT[:, CPB * b: CPB * b + NPAST],
                                 axis=mybir.AxisListType.X)
            pres = compT[:, CPB * b + NPAST: CPB * b + NSEL]
            nc.vector.tensor_tensor_scan(
                gr[:, LOCAL * b: LOCAL * (b + 1)], pres, pres,
                initial=past[:, b:b + 1], op0=ALU.max, op1=ALU.max)

        # ---- phase 3: expand Linear/ReLU/LN ----
        o2_ps = p_o2.tile([OUT_ROWS, DIM], F32)
        nc.tensor.matmul(o2_ps[:], lhsT=gr[:], rhs=W2_sb[:], start=True,
                         stop=False)
        nc.tensor.matmul(o2_ps[:], lhsT=ones_row[0:1, 0:OUT_ROWS], rhs=b2r[:],
                         start=False, stop=True)
        r2 = bigpool.tile([OUT_ROWS, DIM], BF16)
        nc.scalar.activation(r2[:], o2_ps[:], ACTF.Relu)
        st2 = bigpool.tile([OUT_ROWS, 6], F32)
        nc.vector.bn_stats(st2[:], r2[:])
        mv2 = bigpool.tile([OUT_ROWS, 2], F32)
        nc.vector.bn_aggr(mv2[:], st2[:])
        std2 = bigpool.tile([OUT_ROWS, 1], F32)
        nc.scalar.activation(std2[:], mv2[:, 1:2], ACTF.Sqrt,
                             bias=eps_t[0:OUT_ROWS, :])
        rstd2 = bigpool.tile([OUT_ROWS, 1], F32)
        nc.vector.reciprocal(rstd2[:], std2[:])
        negmu2 = bigpool.tile([OUT_ROWS, 1], F32)
        nc.vector.tensor_scalar_mul(negmu2[:], mv2[:, 0:1], -1.0)
        nmr2 = bigpool.tile([OUT_ROWS, 1], F32)
        nc.vector.tensor_tensor(nmr2[:], negmu2[:], rstd2[:], op=ALU.mult)
        o_ln = bigpool.tile([OUT_ROWS, DIM], F32)
        nc.vector.tensor_scalar(o_ln[:], r2[:], rstd2[:], nmr2[:],
                                op0=ALU.mult, op1=ALU.add)
        nc.sync.dma_start(out_d[:].rearrange("b t d -> (b t) d"), o_ln[:])

    nc.compile()
    return nc


_NC = None


def _get_nc():
    global _NC
    if _NC is None:
        _NC = _build()
    return _NC


def _make_in_maps(obs_frames, W1, b1, W2, b2):
    ident = np.eye(128, dtype=np.float32)
    import ml_dtypes
    bf = ml_dtypes.bfloat16
    w1p = np.concatenate([W1[128 * c:128 * (c + 1)] for c in range(4)],
                         axis=1).astype(bf).copy()
    b1r = np.tile(b1, 4).reshape(1, 4 * REDUC).astype(bf).copy()
    b2r = b2.reshape(1, DIM).astype(bf).copy()
    w2 = np.ascontiguousarray(W2).astype(bf)
    in_maps = []
    for c in range(N_CORES):
        shard = np.ascontiguousarray(obs_frames[BL * c:BL * (c + 1)],
                                     dtype=np.float32)
        in_maps.append({"obs": shard, "ident": ident, "w1p": w1p, "w2": w2,
                        "b1r": b1r, "b2r": b2r})
    return in_maps


def _run(obs_frames, W1, b1, g1, beta1, W2, b2, g2, beta2, trace=False):
    assert np.allclose(np.asarray(g1), 1.0) and np.allclose(np.asarray(beta1), 0.0)
    assert np.allclose(np.asarray(g2), 1.0) and np.allclose(np.asarray(beta2), 0.0)
    nc = _get_nc()
    in_maps = _make_in_maps(np.asarray(obs_frames), np.asarray(W1),
                            np.asarray(b1), np.asarray(W2), np.asarray(b2))
    res = run_bass_kernel_spmd(nc, in_maps, list(range(N_CORES)), trace=trace)
    out = np.concatenate([res.results[i]["out"] for i in range(N_CORES)], axis=0)
    return out.astype(np.float32), res


def kernel(obs_frames, W1, b1, g1, beta1, W2, b2, g2, beta2):
    out, _ = _run(obs_frames, W1, b1, g1, beta1, W2, b2, g2, beta2, trace=False)
    return out


def kernel_traced(**inputs):
    return _run(**inputs, trace=True)
